# revision 1
# baseline (speedup 1.0000x reference)
"""Attention-Augmented Conv2D fused Bass kernel for 8 trn2 NeuronCores.

Problem (hardcoded): x [4,64,32,32], NH=8, DK=DV=64, FILTERS=128 -> out [4,128,32,32].
Sharding: core c -> batch b=c//2, head-group g=c%2 (heads 4g..4g+4).
Each core produces:
  o_conv [64,512]  : conv1x1 output for its batch, positions [512g, 512g+512)
  o_attn [64,1024] : partial attn-out conv over its 4 heads (bias only on g==0)
Host gather: conv halves concatenated, attn partials summed per batch.

All projections fold their bias via a ones-row appended to x (x_aug [65,1024]).
Relative-position logits are folded into the single logits matmul:
  logitsT[k,q] = KA[:,k] . QA[:,q]  with K-dim 72:
    KA = [K_h (8) ; D_w (32, k%32 indicator) ; D_h (32, k//32 indicator)]
    QA = [Q_h (8) ; patwT (32) ; pathT (32)]
where patwT[wk,q] = Q[q].rel_w[wk - y_q + 31] etc. come from a projection
pfT = ([rel_w;rel_h] @ Wq_scaled) @ x_aug  followed by a DRAM-roundtrip
shifted-gather DMA. Softmax skips max-subtraction (logits are O(few)); the
denominator comes from a ones-column appended to V^T, so one PE matmul per
tile produces both P@V and the softmax denominator.
"""
import sys
import numpy as np

sys.path.insert(0, '/opt/trn_rl_repo')

NH, DK, DV, FILTERS = 8, 64, 64, 128
B, C, H, W = 4, 64, 32, 32
HW = H * W
dkh = DK // NH
SCALE = dkh ** -0.5
N_CORES = 8


def _build_bass(debug=False):
    import concourse.bass as bass
    import concourse.bacc as bacc
    import concourse.mybir as mybir
    import concourse.tile as tile

    f32 = mybir.dt.float32
    bf16 = mybir.dt.bfloat16
    AF = mybir.ActivationFunctionType

    nc = bacc.Bacc()

    xplus = nc.dram_tensor("xplus", [65, HW + 512 + 64], f32, kind="ExternalInput")
    wcat = nc.dram_tensor("wcat", [65, 604], bf16, kind="ExternalInput")
    wtail = nc.dram_tensor("wtail", [128, 192], f32, kind="ExternalInput")
    dconst = nc.dram_tensor("dconst", [64, HW], bf16, kind="ExternalInput")
    o_conv = nc.dram_tensor("o_conv", [64, 512], f32, kind="ExternalOutput")
    o_attn = nc.dram_tensor("o_attn", [64, HW], f32, kind="ExternalOutput")
    scratch = nc.dram_tensor("pat_scratch", [4, 128, HW], bf16)
    scratch_w = nc.dram_tensor("pat_scratch_w", [4, 128, HW], bf16)
    if debug:
        d_qk = nc.dram_tensor("d_qk", [64, HW], bf16, kind="ExternalOutput")
        d_vt = nc.dram_tensor("d_vt", [128, 8, 36], bf16, kind="ExternalOutput")
        d_pf0 = nc.dram_tensor("d_pf0", [126, HW], bf16, kind="ExternalOutput")
        d_ka0 = nc.dram_tensor("d_ka0", [72, HW], bf16, kind="ExternalOutput")
        d_qa0 = nc.dram_tensor("d_qa0", [72, HW], bf16, kind="ExternalOutput")
        d_pt00 = nc.dram_tensor("d_pt00", [128, HW], bf16, kind="ExternalOutput")
        d_attn = nc.dram_tensor("d_attn", [128, HW], f32, kind="ExternalOutput")
        d_attnn = nc.dram_tensor("d_attnn", [128, HW], f32, kind="ExternalOutput")

    with tile.TileContext(nc) as tc:
        with (
            tc.tile_pool(name="const", bufs=1) as constp,
            tc.tile_pool(name="proj", bufs=1) as projp,
            tc.tile_pool(name="kaqa", bufs=8) as kaqap,
            tc.tile_pool(name="pf", bufs=2) as pfp,
            tc.tile_pool(name="pt", bufs=4) as ptp,
            tc.tile_pool(name="attn", bufs=2) as attnp,
            tc.tile_pool(name="outp", bufs=2) as outp,
            tc.tile_pool(name="ps2", bufs=4, space="PSUM") as ps2,
        ):
            # ---- inputs to SBUF (consolidated) ----
            xplus_sb = constp.tile([65, HW + 512 + 64], f32)
            nc.sync.dma_start(out=xplus_sb, in_=xplus[:, :])
            x_sb = xplus_sb[:, 0:HW]
            xc_sb = xplus_sb[:, HW:HW + 512]
            wconv_sb = xplus_sb[:, HW + 512:HW + 576]
            wcat_sb = constp.tile([65, 604], bf16)
            nc.sync.dma_start(out=wcat_sb, in_=wcat[:, :])
            wqk_sb = wcat_sb[:, 0:64]
            wva_sb = wcat_sb[:, 64:100]
            wpat_sb = wcat_sb[:, 100:604].rearrange("c (h m) -> c h m", h=4)
            wtail_sb = constp.tile([128, 192], f32)
            nc.sync.dma_start(out=wtail_sb, in_=wtail[:, :])
            wattn_sb = wtail_sb[:, 0:64]
            bconst_sb = wtail_sb[:, 64:192]

            # ---- phase 1: projections ----
            x_bf = projp.tile([65, HW], bf16)
            nc.vector.tensor_copy(out=x_bf, in_=x_sb)
            qk_ps = ps2.tile([64, HW], f32, tag="ps2")
            for qc in range(2):
                nc.tensor.matmul(qk_ps[:, 512 * qc:512 * qc + 512],
                                 wqk_sb,
                                 x_bf[:, 512 * qc:512 * qc + 512])
            qk_bf = projp.tile([64, HW], bf16)
            nc.scalar.activation(qk_bf, qk_ps[:, :], AF.Copy)
            if debug:
                nc.sync.dma_start(out=d_qk[:, :], in_=qk_bf)

            # ---- phase 2/3: per-head rel patterns + KA/QA assembly ----
            KA, QA = [], []
            for i in range(4):
                pf_ps = ps2.tile([126, HW], f32, tag="ps2")
                for qc in range(2):
                    nc.tensor.matmul(pf_ps[:, 512 * qc:512 * qc + 512],
                                     wpat_sb[:, i, :],
                                     x_bf[:, 512 * qc:512 * qc + 512])
                pf_sb = pfp.tile([126, HW], bf16)
                nc.vector.tensor_copy(out=pf_sb, in_=pf_ps[:, :])
                nc.sync.dma_start(out=scratch[i, 0:126, :], in_=pf_sb)
                # q'-permuted projection (q' = 32*y + u) for the patw gather
                xp = x_bf.rearrange("c (u y) -> c y u", y=32)
                pfw_ps = ps2.tile([126, HW], f32, tag="ps2")
                for qc in range(2):
                    nc.tensor.matmul(pfw_ps[:, 512 * qc:512 * qc + 512],
                                     wpat_sb[:, i, :],
                                     xp[:, 16 * qc:16 * qc + 16, :])
                pfw_sb = pfp.tile([126, HW], bf16, tag="pfw")
                nc.scalar.activation(pfw_sb, pfw_ps[:, :], AF.Copy)
                nc.sync.dma_start(out=scratch_w[i, 0:126, :], in_=pfw_sb)

                ka = kaqap.tile([72, HW], bf16, tag="ka")
                qa = kaqap.tile([72, HW], bf16, tag="qa")
                KA.append(ka)
                QA.append(qa)
                nc.gpsimd.dma_start(out=ka[64:72, :],
                                    in_=qk_bf[32 + 8 * i:32 + 8 * i + 8, :])
                nc.gpsimd.dma_start(out=qa[64:72, :],
                                    in_=qk_bf[8 * i:8 * i + 8, :])
                nc.scalar.dma_start(out=ka[0:64, :], in_=dconst[:, :])
                # patwT[wk, q' = 32y+u]: scratch_w addr = base + 31744 + 1024*wk - 992*y + u
                tmp_ym = pfp.tile([32, HW], bf16, tag="tmp_ym")
                src_w = bass.AP(scratch_w, i * 128 * HW + 31744,
                                [[1024, 32], [-992, 32], [1, 32]])
                nc.sync.dma_start(
                    out=tmp_ym.rearrange("p (y u) -> p y u", y=32), in_=src_w)
                # unscramble q' -> q: qa[8+wk, 32u+y] = tmp_ym[wk, 32y+u]
                nc.vector.tensor_copy(
                    out=qa[0:32, :].rearrange("p (u y) -> p u y", y=32),
                    in_=tmp_ym.rearrange("p (y u) -> p u y", u=32))
                # pathT[hk, q]: addr = base + 96256 + 1024*hk - 3968*u1 - 992*u0 + y
                dst_h = qa[32:64, :].rearrange("p (a b c) -> p a b c",
                                               a=8, b=4, c=32)
                src_h = bass.AP(scratch, i * 128 * HW + 96256,
                                [[1024, 32], [-3968, 8], [-992, 4], [1, 32]])
                nc.sync.dma_start(out=dst_h, in_=src_h)
                if debug and i == 0:
                    nc.sync.dma_start(out=d_pf0[:, :], in_=pf_sb)
                    nc.sync.dma_start(out=d_ka0[:, :], in_=ka)
                    nc.sync.dma_start(out=d_qa0[:, :], in_=qa)

            vt_ps = ps2.tile([128, 8, 36], f32, tag="ps2")
            for kt in range(8):
                nc.tensor.matmul(vt_ps[:, kt, :],
                                 x_bf[:, 128 * kt:128 * kt + 128],
                                 wva_sb)
            vt_sb = projp.tile([128, 8, 36], bf16)
            nc.scalar.activation(vt_sb, vt_ps[:, :, :], AF.Copy)
            if debug:
                nc.sync.dma_start(out=d_vt[:, :, :], in_=vt_sb)

            conv_ps = ps2.tile([64, 512], f32, tag="ps2")
            nc.tensor.matmul(conv_ps[:, :], wconv_sb, xc_sb)
            conv_sb = outp.tile([64, 512], f32, tag="oconv")
            nc.scalar.activation(conv_sb, conv_ps[:, :], AF.Copy)
            nc.sync.dma_start(out=o_conv[:, :], in_=conv_sb)

            # ---- phase 4: attention main loop ----
            pv_ps = ps2.tile([128, HW], f32, tag="ps2")
            nc.vector.memset(pv_ps[:, :], 1.0)
            seq = [(i, kt) for i in range(4) for kt in range(8)]

            def emit_lg(i, kt):
                lg_ps = ps2.tile([128, HW], f32, tag="ps2")
                for qc in range(2):
                    nc.tensor.matmul(
                        lg_ps[:, 512 * qc:512 * qc + 512],
                        KA[i][:, 128 * kt:128 * kt + 128],
                        QA[i][:, 512 * qc:512 * qc + 512])
                return lg_ps

            lg_tiles = {seq[0]: emit_lg(*seq[0])}
            for j, (i, kt) in enumerate(seq):
                if j + 1 < len(seq):
                    lg_tiles[seq[j + 1]] = emit_lg(*seq[j + 1])
                lg_ps = lg_tiles.pop((i, kt))
                pt = ptp.tile([128, HW], bf16)
                nc.scalar.activation(pt, lg_ps[:, :], AF.Exp)
                if debug and i == 0 and kt == 0:
                    nc.sync.dma_start(out=d_pt00[:, :], in_=pt)
                for qc in range(2):
                    nc.tensor.matmul(
                        pv_ps[32 * i:32 * i + 9, 512 * qc:512 * qc + 512],
                        vt_sb[:, kt, 9 * i:9 * i + 9],
                        pt[:, 512 * qc:512 * qc + 512],
                        start=(kt == 0), stop=(kt == 7),
                        tile_position=(0, 32 * i))

            recb_sb = attnp.tile([128, HW], f32, tag="recb")
            attn_n = attnp.tile([128, HW], f32, tag="attn_n")
            if debug:
                d_attn_sb = attnp.tile([128, HW], f32, tag="dbg")
            rp = attnp.tile([128, HW], f32, tag="rp")
            nc.vector.reciprocal(out=rp, in_=pv_ps[:, :])
            recb_ps = ps2.tile([128, HW], f32, tag="ps2")
            for qc in range(2):
                nc.tensor.matmul(recb_ps[:, 512 * qc:512 * qc + 512],
                                 bconst_sb,
                                 rp[:, 512 * qc:512 * qc + 512])
            nc.vector.tensor_copy(out=recb_sb, in_=recb_ps[:, :])
            nc.vector.tensor_mul(attn_n, pv_ps[:, :], recb_sb)
            if debug:
                nc.vector.tensor_copy(out=d_attn_sb, in_=attn_n)
                nc.sync.dma_start(out=d_attn[:, :], in_=d_attn_sb)
                nc.sync.dma_start(out=d_attnn[:, :], in_=d_attn_sb)

            oat_ps = ps2.tile([64, HW], f32, tag="ps2")
            for qc in range(2):
                nc.tensor.matmul(oat_ps[:, 512 * qc:512 * qc + 512],
                                 wattn_sb,
                                 attn_n[:, 512 * qc:512 * qc + 512])
            oat_sb = outp.tile([64, HW], f32, tag="oattn")
            nc.vector.tensor_copy(out=oat_sb, in_=oat_ps[:, :])
            nc.sync.dma_start(out=o_attn[:, :], in_=oat_sb)

    nc.compile()
    return nc


def _host_prep(inputs):
    import ml_dtypes
    x = np.ascontiguousarray(inputs['x'], np.float32)
    w_qkv = np.ascontiguousarray(inputs['w_qkv'].reshape(2 * DK + DV, C), np.float32)
    b_qkv = np.ascontiguousarray(inputs['b_qkv'], np.float32)
    w_conv = np.ascontiguousarray(inputs['w_conv'].reshape(FILTERS - DV, C), np.float32)
    b_conv = np.ascontiguousarray(inputs['b_conv'], np.float32)
    w_attn = np.ascontiguousarray(inputs['w_attn'].reshape(DV, DV), np.float32)
    b_attn = np.ascontiguousarray(inputs['b_attn'], np.float32)
    rel_h = np.ascontiguousarray(inputs['key_rel_h'], np.float32)
    rel_w = np.ascontiguousarray(inputs['key_rel_w'], np.float32)
    relcat = np.concatenate([rel_w, rel_h], 0)  # [126, 8]

    kk = np.arange(HW)
    DCmat = np.zeros((64, HW), np.float32)
    DCmat[:32] = (kk[None, :] % 32 == np.arange(32)[:, None])
    DCmat[32:] = (kk[None, :] // 32 == np.arange(32)[:, None])
    DCmat = DCmat.astype(ml_dtypes.bfloat16)
    BC = np.zeros((128, 128), np.float32)
    for p in range(128):
        BC[32 * (p // 32), p] = 1.0

    wconv_aug = np.ascontiguousarray(
        np.concatenate([w_conv, b_conv[:, None]], 1).T)

    in_maps = []
    for c in range(N_CORES):
        b, g = c // 2, c % 2
        heads = [4 * g + i for i in range(4)]
        x_aug = np.concatenate([x[b].reshape(C, HW),
                                np.ones((1, HW), np.float32)], 0)
        wq = w_qkv[32 * g:32 * g + 32] * SCALE
        bq = b_qkv[32 * g:32 * g + 32] * SCALE
        wk = w_qkv[64 + 32 * g:64 + 32 * g + 32]
        bk = b_qkv[64 + 32 * g:64 + 32 * g + 32]
        wqk_aug = np.ascontiguousarray(np.concatenate(
            [np.concatenate([wq, wk], 0),
             np.concatenate([bq, bk], 0)[:, None]], 1).T)
        wva_m = np.zeros((65, 36), np.float32)
        wpat_m = np.zeros((65, 4, 126), np.float32)
        for i, h in enumerate(heads):
            wv = w_qkv[128 + 8 * h:128 + 8 * h + 8]
            bv = b_qkv[128 + 8 * h:128 + 8 * h + 8]
            wva_m[64, 9 * i] = 1.0
            wva_m[:64, 9 * i + 1:9 * i + 9] = wv.T
            wva_m[64, 9 * i + 1:9 * i + 9] = bv
            wq_h = w_qkv[8 * h:8 * h + 8] * SCALE
            bq_h = b_qkv[8 * h:8 * h + 8] * SCALE
            wpat_m[:64, i, :] = (relcat @ wq_h).T
            wpat_m[64, i, :] = relcat @ bq_h
        wattn_aug = np.zeros((128, 64), np.float32)
        for i, h in enumerate(heads):
            wattn_aug[32 * i + 1:32 * i + 9] = w_attn[:, 8 * h:8 * h + 8].T
        if g == 0:
            wattn_aug[0] += b_attn
        # attn_n rows: pair p head ii at 64p + 32ii -- same as 32i ordering
        xplus = np.concatenate(
            [x_aug, x_aug[:, 512 * g:512 * g + 512], wconv_aug], 1)
        wcat = np.concatenate(
            [wqk_aug, wva_m, wpat_m.reshape(65, 504)], 1)
        wtail = np.concatenate([wattn_aug, BC], 1)
        in_maps.append({
            'xplus': np.ascontiguousarray(xplus, np.float32),
            'wcat': np.ascontiguousarray(wcat.astype(ml_dtypes.bfloat16)),
            'wtail': np.ascontiguousarray(wtail, np.float32),
            'dconst': DCmat,
        })
    return in_maps


_CACHED = {}


def kernel(**inputs):
    from concourse.bass_utils import run_bass_kernel_spmd
    if 'nc' not in _CACHED:
        _CACHED['nc'] = _build_bass()
    nc = _CACHED['nc']
    in_maps = _host_prep(inputs)
    res = run_bass_kernel_spmd(nc, in_maps, core_ids=list(range(N_CORES)))
    out = np.zeros((B, FILTERS, HW), np.float32)
    for c in range(N_CORES):
        b, g = c // 2, c % 2
        out[b, :64, 512 * g:512 * g + 512] = res.results[c]['o_conv']
        out[b, 64:] += res.results[c]['o_attn']
    return out.reshape(B, FILTERS, H, W)

